# revision 7
# baseline (speedup 1.0000x reference)
"""Grouped SwiGLU expert FFN (MoE) on 8 Trainium2 NeuronCores.

Expert parallelism: expert e's weights + its (pre-sorted) token slice go to
core e. Each core runs x@w1, x@w3, silu/mul, h@w2 for its 8192 tokens.

All matmul operands are bf16 (PE full rate, FWL weight loads overlap with
streaming — fp32r weight loads are self-loading and serialize, which cost
~270us in the fp32r version). x is transposed + cast on the host so the
kernel does zero PE transposes; weights are cast to bf16 on the host.

Math per core (dims: t=tokens, i=dim_in, j=dim_hid, o=dim_in):
  mm1/mm3: psum[j,t] += lhsT=w{1,3}[i_chunk, j_chunk] (stationary),
           rhs=xT[i_chunk, t_sb] (moving, 512 wide) -> h1T/h3T.
  SwiGLU:  hT = silu(h1T) * h3T  (ACT Silu + DVE mul, PSUM eviction fused,
           bf16 out).
  mm2:     lhsT=hT[j_chunk, t_chunk] (stationary), rhs=w2[j_chunk, o_half]
           (moving, 512 wide) -> psum[t,o] natural-layout, bf16 eviction.

Output is bf16 on device, upcast to f32 on the host (adds ~2e-3 rel err,
total ~4.4e-3 vs the 2e-2 gate).
"""

import sys

sys.path.insert(0, "/opt/trn_rl_repo")

import numpy as np
import ml_dtypes

N_CORES = 8
D = 1024  # dim_in
H = 1024  # dim_hid
P = 128
TS = 512  # token superblock (psum bank = 512 fp32 cols)

BF16 = ml_dtypes.bfloat16

_CACHE = {}


def _build(tok):
    import concourse.bacc as bacc
    import concourse.tile as tile
    from concourse import mybir

    dt = mybir.dt
    AF = mybir.ActivationFunctionType
    bf16 = dt.bfloat16
    f32 = dt.float32

    assert tok % TS == 0
    n_sb = tok // TS
    n_i = D // P   # 8 contraction chunks for mm1/mm3
    n_j = H // P   # 8 contraction chunks for mm2
    n_tc = TS // P  # 4 token chunks (128) per superblock

    nc = bacc.Bacc(trn_type="TRN2", target_bir_lowering=False)
    xT_h = nc.dram_tensor("xT", [D, tok], bf16, kind="ExternalInput")
    w1_h = nc.dram_tensor("w1", [D, H], bf16, kind="ExternalInput")
    w2_h = nc.dram_tensor("w2", [H, D], bf16, kind="ExternalInput")
    w3_h = nc.dram_tensor("w3", [D, H], bf16, kind="ExternalInput")
    out_h = nc.dram_tensor("out", [tok, D], bf16, kind="ExternalOutput")

    with tile.TileContext(nc) as tc:
        with (
            tc.tile_pool(name="wpool", bufs=1) as wpool,
            tc.tile_pool(name="xtpool", bufs=2) as xtpool,
            tc.tile_pool(name="htpool", bufs=2) as htpool,
            tc.tile_pool(name="spool", bufs=3) as spool,
            tc.tile_pool(name="opool", bufs=2) as opool,
            tc.tile_pool(name="pA", bufs=2, space="PSUM") as pAp,
            tc.tile_pool(name="pB", bufs=2, space="PSUM") as pBp,
            tc.tile_pool(name="pC", bufs=2, space="PSUM") as pCp,
            tc.tile_pool(name="pD", bufs=2, space="PSUM") as pDp,
        ):
            # Resident weights, partition = row-within-chunk: [P, n_chunks, cols]
            w1s = wpool.tile([P, n_i, H], bf16)
            w3s = wpool.tile([P, n_i, H], bf16)
            w2s = wpool.tile([P, n_j, D], bf16)

            xT_r = xT_h[:, :].rearrange("(c p) t -> p c t", p=P)
            o_r = out_h[:, :].rearrange("(b c p) d -> b p c d", p=P, c=n_tc)

            # Startup streaming: first xT superblock lands first (in 4
            # chunks so mm1 i=0 can start early), then w1/w3 column-blocks
            # in j order (mm1/mm3 j consumes them in order), then w2 (only
            # needed once mm2 of superblock 0 starts, ~28us in).
            xT0 = xtpool.tile([P, n_i, TS], bf16)
            w1r = w1_h[:, :].rearrange("(c p) h -> p c h", p=P)
            w3r = w3_h[:, :].rearrange("(c p) h -> p c h", p=P)
            # Critical path first: the j=0 column-blocks of w1/w3 and the
            # first xT chunks gate the first matmuls.
            nc.sync.dma_start(out=xT0[:, 0:1, :], in_=xT_r[:, 0:1, 0:TS])
            nc.sync.dma_start(out=w1s[:, :, 0:P], in_=w1r[:, :, 0:P])
            nc.sync.dma_start(out=w3s[:, :, 0:P], in_=w3r[:, :, 0:P])
            for c in range(1, n_i):
                nc.sync.dma_start(
                    out=xT0[:, c:c + 1, :], in_=xT_r[:, c:c + 1, 0:TS]
                )
            for j in range(1, n_j):
                nc.sync.dma_start(
                    out=w1s[:, :, j * P:(j + 1) * P],
                    in_=w1r[:, :, j * P:(j + 1) * P],
                )
                nc.sync.dma_start(
                    out=w3s[:, :, j * P:(j + 1) * P],
                    in_=w3r[:, :, j * P:(j + 1) * P],
                )
            nc.sync.dma_start(
                out=w2s, in_=w2_h[:, :].rearrange("(c p) h -> p c h", p=P)
            )

            for b in range(n_sb):
                # ---- load xT superblock [P(=i in chunk), n_i, TS]
                if b == 0:
                    xT = xT0
                else:
                    xT = xtpool.tile([P, n_i, TS], bf16)
                    for c in range(n_i):
                        nc.sync.dma_start(
                            out=xT[:, c:c + 1, :],
                            in_=xT_r[:, c:c + 1, b * TS:(b + 1) * TS],
                        )

                # ---- mm1/mm3 + SwiGLU -> hT [P(=j in chunk), n_j, TS] bf16
                hT = htpool.tile([P, n_j, TS], bf16)
                for j in range(n_j):
                    pA = pAp.tile([P, TS], f32)
                    pB = pBp.tile([P, TS], f32)
                    for i in range(n_i):
                        nc.tensor.matmul(
                            pA, w1s[:, i, j * P:(j + 1) * P], xT[:, i, :],
                            start=(i == 0), stop=(i == n_i - 1),
                        )
                    for i in range(n_i):
                        nc.tensor.matmul(
                            pB, w3s[:, i, j * P:(j + 1) * P], xT[:, i, :],
                            start=(i == 0), stop=(i == n_i - 1),
                        )
                    s1 = spool.tile([P, TS], bf16)
                    nc.scalar.activation(s1, pA, AF.Silu)
                    nc.vector.tensor_mul(hT[:, j, :], pB, s1)

                # ---- mm2 -> natural-layout out superblock, bf16
                o_sb = opool.tile([P, n_tc, D], bf16)
                for t in range(n_tc):
                    pC = pCp.tile([P, 512], f32)
                    pD = pDp.tile([P, 512], f32)
                    for j in range(n_j):
                        nc.tensor.matmul(
                            pC,
                            hT[:, j, t * P:(t + 1) * P],
                            w2s[:, j, 0:512],
                            start=(j == 0), stop=(j == n_j - 1),
                        )
                        nc.tensor.matmul(
                            pD,
                            hT[:, j, t * P:(t + 1) * P],
                            w2s[:, j, 512:1024],
                            start=(j == 0), stop=(j == n_j - 1),
                        )
                    nc.vector.tensor_copy(o_sb[:, t, 0:512], pC)
                    nc.vector.tensor_copy(o_sb[:, t, 512:1024], pD)
                    nc.sync.dma_start(out=o_r[b][:, t, :], in_=o_sb[:, t, :])

    nc.compile()
    return nc


def _get_nc(tok):
    if tok not in _CACHE:
        _CACHE[tok] = _build(tok)
    return _CACHE[tok]


def _prep_in_maps(x, w1, w2, w3, sizes, pad):
    """Host-side shard + layout: per-expert token slice, transposed + bf16."""
    offs = np.concatenate([[0], np.cumsum(sizes)])
    in_maps = []
    for e in range(N_CORES):
        xe = np.asarray(x[offs[e]:offs[e + 1]], dtype=np.float32)
        xeT = np.zeros((D, pad), dtype=BF16)
        xeT[:, : xe.shape[0]] = xe.T.astype(BF16)
        in_maps.append(
            {
                "xT": xeT,
                "w1": np.asarray(w1[e], dtype=np.float32).astype(BF16),
                "w2": np.asarray(w2[e], dtype=np.float32).astype(BF16),
                "w3": np.asarray(w3[e], dtype=np.float32).astype(BF16),
            }
        )
    return in_maps


def kernel(x, w1, w2, w3, m_sizes):
    from concourse.bass_utils import run_bass_kernel_spmd

    sizes = np.asarray(m_sizes).astype(np.int64)
    offs = np.concatenate([[0], np.cumsum(sizes)])
    n_exp = sizes.shape[0]
    assert n_exp == N_CORES

    pad = int(max(int(sizes.max()), TS))
    pad = ((pad + TS - 1) // TS) * TS
    nc = _get_nc(pad)

    in_maps = _prep_in_maps(x, w1, w2, w3, sizes, pad)
    r = run_bass_kernel_spmd(nc, in_maps, core_ids=list(range(N_CORES)))
    out = np.concatenate(
        [
            np.asarray(r.results[e]["out"][: sizes[e]]).astype(np.float32)
            for e in range(N_CORES)
        ],
        axis=0,
    )
    return out
